# revision 27
# baseline (speedup 1.0000x reference)
"""LSTM encoder with EOS-freeze for Trainium2, data-parallel over batch on 8 cores.

Strategy
--------
Inputs are one-hot, so x @ Wi is a row-gather of Wi done with indirect DMA on
device. The recurrent h @ Wh runs on the tensor engine with Wh as 64 fp16
[128,128] stationary tiles and h.T chunks as the [128,16] moving operand,
producing z transposed: per-gate PSUM banks [128 partitions = feature % 128,
16*tile + b].

The per-step critical path is (h-dependent matmul burst) -> (gate tail) -> h.
To hide the tail: gates are reordered (g, i, f, o) host-side and the matmuls
run tile-outer, so each gate's sigmoid + DVE ops overlap the later gates'
matmuls; each gate gets a full PSUM bank so those reads never serialize the
stream. The g block is pre-scaled by 2 so tanh(g) = 2*sigmoid(2g) - 1 stays
on the sigmoid table, and only sigma_o || (c = A+B) -> tanh(c) -> h remain
after the burst. Next-step x-transposes are emitted before this step's tail
so the PE runs them in the tail window.

The EOS freeze is handled without any per-step masking: sequences are
independent, so the kernel runs the unmasked recurrence and streams per-step
(c, h) snapshots to DRAM; the frozen value for sequence b is the snapshot at
its first-EOS step, selected during unshard.
"""

import numpy as np

try:
    import concourse  # noqa: F401
except ImportError:
    import sys

    sys.path.insert(0, "/opt/trn_rl_repo")

from contextlib import ExitStack

import concourse.bass as bass
import concourse.tile as tile
from concourse import bacc
from concourse import mybir
from concourse.bass import ds
from concourse.bass_utils import run_bass_kernel_spmd

dt = mybir.dt
Alu = mybir.AluOpType
Act = mybir.ActivationFunctionType

EOS_ID = 1
HID = 512
BATCH, SEQ, VOCAB = 128, 256, 1024
GATES = 4 * HID  # 2048
NCORES = 8
BLOC = BATCH // NCORES  # 16 sequences per core
NT = GATES // 128  # 16 feature tiles of z
NK = HID // 128  # 4 contraction chunks
BODY = 16  # steps per For_i iteration

# Collect profiling info when True (set by test.py; adds trace overhead).
TRACE = False
LAST_RESULTS = None  # BassKernelResults of the last run, for test.py

_PROGRAM = None


def _build_program(seq=SEQ, body=BODY):
    nc = bacc.Bacc("TRN2", debug=False, detect_race_conditions=False)

    wi = nc.declare_dram_parameter("wi", [VOCAB, GATES], dt.float16, isOutput=False)
    ident = nc.declare_dram_parameter("ident", [BLOC, BLOC], dt.float16, isOutput=False)
    wh = nc.declare_dram_parameter("wh", [128, NK * NT * 128], dt.float16, isOutput=False)
    tok = nc.declare_dram_parameter("tok", [BLOC, seq + body], dt.int32, isOutput=False)
    c_traj = nc.declare_dram_parameter("c_traj", [seq * 128, 64], dt.float32, isOutput=True)
    h_traj = nc.declare_dram_parameter("h_traj", [seq * 128, 64], dt.float16, isOutput=True)

    with tile.TileContext(nc) as tc, ExitStack() as ctx:
        pool = lambda name, bufs, **kw: ctx.enter_context(
            tc.tile_pool(name=name, bufs=bufs, **kw)
        )
        whp = pool("whp", 1)
        tokp = pool("tokp", 1)
        stp = pool("stp", 1)
        hp = pool("hp", 1)
        cp = pool("cp", 1)
        zp_pool = pool("zp", 2, space="PSUM")
        sp = pool("sp", 3)
        gp = pool("gp", 3)
        ap_ = pool("ap", 3)
        bp = pool("bp", 3)
        s2p = pool("s2p", 2)
        tp = pool("tp", 3)

        wh_sb = whp.tile([128, NK * NT * 128], dt.float16, name="wh_sb")
        nc.sync.dma_start(out=wh_sb[:], in_=wh[:, :])
        tok_cur = tokp.tile([BLOC, body], dt.int32, name="tok_cur")
        nc.sync.dma_start(out=tok_cur[:], in_=tok[:, 0:body])
        id_sb = tokp.tile([BLOC, BLOC], dt.float16, name="id_sb")
        nc.sync.dma_start(out=id_sb[:], in_=ident[:, :])

        ST = [stp.tile([BLOC, GATES], dt.float16, name=f"st{s}", tag=f"st{s}") for s in range(body)]
        H = [hp.tile([128, 64], dt.float16, name=f"h{s}", tag=f"h{s}") for s in range(body)]
        C = [cp.tile([128, 64], dt.float32, name=f"c{s}", tag=f"c{s}") for s in range(body)]

        nc.gpsimd.memset(H[body - 1][:], 0.0)
        nc.gpsimd.memset(C[body - 1][:], 0.0)
        for s in range(body):
            # init shadow coverage; real values come from the indirect gathers
            nc.gpsimd.memset(ST[s][:], 0.0)

        def gather_xp(s):
            # Gather BLOC wi rows (one per sequence) for one timestep into
            # ST[s][b, :] — row-per-partition, the DGE-supported shape.
            # tok_cur always holds the token column for the block being
            # prefetched, so the offset AP stays static.
            nc.gpsimd.indirect_dma_start(
                out=ST[s][:],
                out_offset=None,
                in_=wi[:, :],
                in_offset=bass.IndirectOffsetOnAxis(ap=tok_cur[:, s : s + 1], axis=0),
            )

        for s in range(body):
            gather_xp(s)

        def alloc_z():
            # One full PSUM bank per gate (a tile smaller than a bank shares
            # it, and start=True clears whole banks, which would serialize
            # steps): a finished gate's sigmoid read never blocks the
            # still-streaming matmuls of later gates.
            return [
                zp_pool.tile([128, 512], dt.float32, name=f"z{g}", tag=f"z{g}")
                for g in "gifo"
            ]

        def xp_matmuls(Z, s):
            # x@Wi enters PSUM via PE transpose of the gathered rows: these
            # matmuls need no h, so they overlap the previous step's tail.
            for t in range(NT):
                # start=True on each tile's first matmul clears that bank
                # region's has_written bits; later matmuls accumulate.
                nc.tensor.matmul(
                    out=Z[t // 4][:, 16 * (t % 4) : 16 * (t % 4) + 16],
                    lhsT=ST[s][:, 128 * t : 128 * t + 128],
                    rhs=id_sb[:],
                    start=(t % 4 == 0),
                    stop=False,
                )

        def step(iv, s, Z, Znext):
            hprev = H[(s - 1) % body]
            cprev = C[(s - 1) % body]
            # Gate order along z-features is (g, f, i, o), four tiles each.
            # Tile-outer / k-inner matmul order completes each gate's PSUM
            # slice early, so its sigmoid + DVE ops run while later gates'
            # matmuls still stream; only sigma_o || c-add -> tanh -> h remain
            # after the burst. The g block is pre-scaled x2 host-side so
            # tanh(g) = 2*sigmoid(2g) - 1 reuses the sigmoid table.
            S = sp.tile([128, 256], dt.float16, name="S", tag="S")
            TG = gp.tile([128, 64], dt.float16, name="TG", tag="TG")
            A = ap_.tile([128, 64], dt.float32, name="A", tag="A")
            B = bp.tile([128, 64], dt.float32, name="B", tag="B")
            T = tp.tile([128, 64], dt.float16, name="T", tag="T")
            Sf = s2p.tile([128, 64], dt.float32, name="Sf", tag="Sf")
            cs = C[s]
            for t in range(NT):
                for k in range(NK):
                    nc.tensor.matmul(
                        out=Z[t // 4][:, 16 * (t % 4) : 16 * (t % 4) + 16],
                        lhsT=wh_sb[:, (k * NT + t) * 128 : (k * NT + t) * 128 + 128],
                        rhs=hprev[:, 16 * k : 16 * k + 16],
                        start=False,
                        stop=(t % 4 == 3 and k == NK - 1),
                    )
                if t == 3:  # g tiles 0-3 complete
                    nc.scalar.activation(out=S[:, 0:64], in_=Z[0][:, 0:64], func=Act.Sigmoid)
                    nc.vector.tensor_scalar(out=TG[:], in0=S[:, 0:64], scalar1=2.0,
                                            scalar2=1.0, op0=Alu.mult, op1=Alu.subtract)
                elif t == 7:  # i tiles 4-7 complete
                    nc.scalar.activation(out=S[:, 64:128], in_=Z[1][:, 0:64], func=Act.Sigmoid)
                    nc.vector.tensor_tensor(out=A[:], in0=S[:, 64:128], in1=TG[:], op=Alu.mult)
                elif t == 11:  # f tiles 8-11 complete: c and tanh(c) still
                    # fit inside the o-gate matmuls. sigma_f stays fp32: its
                    # error compounds multiplicatively through c.
                    nc.scalar.activation(out=Sf[:], in_=Z[2][:, 0:64], func=Act.Sigmoid)
                    nc.vector.tensor_tensor(out=B[:], in0=Sf[:], in1=cprev[:], op=Alu.mult)
                    nc.vector.tensor_tensor(out=cs[:], in0=A[:], in1=B[:], op=Alu.add)
                    nc.scalar.activation(out=T[:], in_=cs[:], func=Act.Tanh)
            # Next step's x transposes go here in program order so the PE
            # runs them during this step's tail instead of at the head of the
            # next h-dependent burst.
            if Znext is not None:
                xp_matmuls(Znext, s + 1)
            # o tiles 12-15: only sigma_o -> h remains after the burst
            nc.scalar.activation(out=S[:, 192:256], in_=Z[3][:, 0:64], func=Act.Sigmoid)
            hs = H[s]
            nc.vector.tensor_tensor(out=hs[:], in0=S[:, 192:256], in1=T[:], op=Alu.mult)

            nc.sync.dma_start(out=c_traj[ds((iv + s) * 128, 128), :], in_=cs[:])
            nc.sync.dma_start(out=h_traj[ds((iv + s) * 128, 128), :], in_=hs[:])
            # Prefetch this slot's xp for the next block (the token table is
            # padded so the final block reads harmless extra rows).
            gather_xp(s)

        with tc.For_i(0, seq, body, hint_engines=(mybir.EngineType.PE,), staggered_reset=True) as iv:
            # Stage the NEXT block's token columns; in-loop gathers prefetch
            # for block i+1 while this block computes.
            nc.sync.dma_start(out=tok_cur[:], in_=tok[:, ds(iv + body, body)])
            Z = alloc_z()
            xp_matmuls(Z, 0)
            for s in range(body):
                Znext = alloc_z() if s < body - 1 else None
                step(iv, s, Z, Znext)
                Z = Znext

    nc.finalize()
    return nc


def _get_program():
    global _PROGRAM
    if _PROGRAM is None:
        _PROGRAM = _build_program()
    return _PROGRAM


def _prep_host(inputs, Wi, Wh, b):
    tokens = np.argmax(inputs, axis=-1).astype(np.int32)  # [B, T]
    eos = inputs[:, :, EOS_ID] > 0.5
    any_eos = eos.any(axis=1)
    t_star = np.where(any_eos, eos.argmax(axis=1), SEQ - 1).astype(np.int64)

    # Gate reorder (g, f, i, o): each gate's four z-feature tiles finish
    # early in the tile-outer matmul order, overlapping its tail ops with
    # the remaining matmuls. The g block is pre-scaled x2 so
    # tanh(g) = 2*sigmoid(2g) - 1 reuses the sigmoid table.
    perm = np.concatenate(
        [np.arange(1024, 1536), np.arange(0, 512), np.arange(512, 1024), np.arange(1536, 2048)]
    )
    gate_scale = np.ones((GATES,), np.float32)
    gate_scale[0:512] = 2.0  # g block after permutation
    Wi_re = (Wi.astype(np.float32) + b.astype(np.float32)[None, :])[:, perm] * gate_scale
    Wh_re = Wh.astype(np.float32)[:, perm] * gate_scale

    Wi_dev = np.ascontiguousarray(Wi_re).astype(np.float16)
    # Partition-major: wh[kr, (k*NT+t)*128 + p] = Wh_re[128k+kr, 128t+p]
    Wh_dev = np.ascontiguousarray(
        Wh_re.reshape(NK, 128, NT, 128).transpose(1, 0, 2, 3).reshape(128, NK * NT * 128)
    ).astype(np.float16)
    return tokens, t_star, Wi_dev, Wh_dev


def kernel(inputs, Wi, Wh, b):
    global LAST_RESULTS
    inputs = np.asarray(inputs)
    Wi = np.asarray(Wi)
    Wh = np.asarray(Wh)
    b = np.asarray(b)

    tokens, t_star, Wi_dev, Wh_dev = _prep_host(inputs, Wi, Wh, b)

    in_maps = []
    for n in range(NCORES):
        tokc = tokens[BLOC * n : BLOC * (n + 1)]
        tok_pad = np.concatenate([tokc, np.zeros((BLOC, BODY), np.int32)], axis=1)
        in_maps.append(
            {
                "wi": Wi_dev,
                "wh": Wh_dev,
                "tok": np.ascontiguousarray(tok_pad),
                "ident": np.eye(BLOC, dtype=np.float16),
            }
        )

    nc = _get_program()
    res = run_bass_kernel_spmd(nc, in_maps, list(range(NCORES)), trace=TRACE)
    LAST_RESULTS = res

    c_out = np.zeros((BATCH, HID), np.float32)
    h_out = np.zeros((BATCH, HID), np.float32)
    for n in range(NCORES):
        ct = res.results[n]["c_traj"].reshape(SEQ, 128, 64)
        ht = res.results[n]["h_traj"].reshape(SEQ, 128, 64).astype(np.float32)
        for bl in range(BLOC):
            g = BLOC * n + bl
            t = int(t_star[g])
            c_out[g] = ct[t][:, bl::BLOC].T.reshape(HID)
            h_out[g] = ht[t][:, bl::BLOC].T.reshape(HID)
    return (c_out, h_out)



# revision 28
# speedup vs baseline: 1.0023x; 1.0023x over previous
"""LSTM encoder with EOS-freeze for Trainium2, data-parallel over batch on 8 cores.

Strategy
--------
Inputs are one-hot, so x @ Wi is a row-gather of Wi done with indirect DMA on
device. The recurrent h @ Wh runs on the tensor engine with Wh as 64 fp16
[128,128] stationary tiles and h.T chunks as the [128,16] moving operand,
producing z transposed: per-gate PSUM banks [128 partitions = feature % 128,
16*tile + b].

The per-step critical path is (h-dependent matmul burst) -> (gate tail) -> h.
To hide the tail: gates are reordered (g, i, f, o) host-side and the matmuls
run tile-outer, so each gate's sigmoid + DVE ops overlap the later gates'
matmuls; each gate gets a full PSUM bank so those reads never serialize the
stream. The g block is pre-scaled by 2 so tanh(g) = 2*sigmoid(2g) - 1 stays
on the sigmoid table, and only sigma_o || (c = A+B) -> tanh(c) -> h remain
after the burst. Next-step x-transposes are emitted before this step's tail
so the PE runs them in the tail window.

The EOS freeze is handled without any per-step masking: sequences are
independent, so the kernel runs the unmasked recurrence and streams per-step
(c, h) snapshots to DRAM; the frozen value for sequence b is the snapshot at
its first-EOS step, selected during unshard.
"""

import numpy as np

try:
    import concourse  # noqa: F401
except ImportError:
    import sys

    sys.path.insert(0, "/opt/trn_rl_repo")

from contextlib import ExitStack

import concourse.bass as bass
import concourse.tile as tile
from concourse import bacc
from concourse import mybir
from concourse.bass import ds
from concourse.bass_utils import run_bass_kernel_spmd

dt = mybir.dt
Alu = mybir.AluOpType
Act = mybir.ActivationFunctionType

EOS_ID = 1
HID = 512
BATCH, SEQ, VOCAB = 128, 256, 1024
GATES = 4 * HID  # 2048
NCORES = 8
BLOC = BATCH // NCORES  # 16 sequences per core
NT = GATES // 128  # 16 feature tiles of z
NK = HID // 128  # 4 contraction chunks
BODY = 16  # steps per For_i iteration

# Collect profiling info when True (set by test.py; adds trace overhead).
TRACE = False
LAST_RESULTS = None  # BassKernelResults of the last run, for test.py

_PROGRAM = None


def _build_program(seq=SEQ, body=BODY):
    nc = bacc.Bacc("TRN2", debug=False, detect_race_conditions=False)

    wi = nc.declare_dram_parameter("wi", [VOCAB, GATES], dt.float16, isOutput=False)
    ident = nc.declare_dram_parameter("ident", [BLOC, BLOC], dt.float16, isOutput=False)
    wh = nc.declare_dram_parameter("wh", [128, NK * NT * 128], dt.float16, isOutput=False)
    tok = nc.declare_dram_parameter("tok", [BLOC, seq + body], dt.int32, isOutput=False)
    c_traj = nc.declare_dram_parameter("c_traj", [seq * 128, 64], dt.float32, isOutput=True)
    h_traj = nc.declare_dram_parameter("h_traj", [seq * 128, 64], dt.float16, isOutput=True)

    with tile.TileContext(nc) as tc, ExitStack() as ctx:
        pool = lambda name, bufs, **kw: ctx.enter_context(
            tc.tile_pool(name=name, bufs=bufs, **kw)
        )
        whp = pool("whp", 1)
        tokp = pool("tokp", 1)
        stp = pool("stp", 1)
        hp = pool("hp", 1)
        cp = pool("cp", 1)
        zp_pool = pool("zp", 2, space="PSUM")
        sp = pool("sp", 3)
        gp = pool("gp", 3)
        ap_ = pool("ap", 3)
        bp = pool("bp", 3)
        s2p = pool("s2p", 2)
        tp = pool("tp", 3)

        wh_sb = whp.tile([128, NK * NT * 128], dt.float16, name="wh_sb")
        nc.sync.dma_start(out=wh_sb[:], in_=wh[:, :])
        tok_cur = tokp.tile([BLOC, body], dt.int32, name="tok_cur")
        nc.sync.dma_start(out=tok_cur[:], in_=tok[:, 0:body])
        id_sb = tokp.tile([BLOC, BLOC], dt.float16, name="id_sb")
        nc.sync.dma_start(out=id_sb[:], in_=ident[:, :])

        ST = [stp.tile([BLOC, GATES], dt.float16, name=f"st{s}", tag=f"st{s}") for s in range(body)]
        H = [hp.tile([128, 64], dt.float16, name=f"h{s}", tag=f"h{s}") for s in range(body)]
        C = [cp.tile([128, 64], dt.float32, name=f"c{s}", tag=f"c{s}") for s in range(body)]

        nc.gpsimd.memset(H[body - 1][:], 0.0)
        nc.gpsimd.memset(C[body - 1][:], 0.0)
        for s in range(body):
            # init shadow coverage; real values come from the indirect gathers
            nc.gpsimd.memset(ST[s][:], 0.0)

        def gather_xp(s):
            # Gather BLOC wi rows (one per sequence) for one timestep into
            # ST[s][b, :] — row-per-partition, the DGE-supported shape.
            # tok_cur always holds the token column for the block being
            # prefetched, so the offset AP stays static.
            nc.gpsimd.indirect_dma_start(
                out=ST[s][:],
                out_offset=None,
                in_=wi[:, :],
                in_offset=bass.IndirectOffsetOnAxis(ap=tok_cur[:, s : s + 1], axis=0),
            )

        for s in range(body):
            gather_xp(s)

        def alloc_z():
            # One full PSUM bank per gate (a tile smaller than a bank shares
            # it, and start=True clears whole banks, which would serialize
            # steps): a finished gate's sigmoid read never blocks the
            # still-streaming matmuls of later gates.
            return [
                zp_pool.tile([128, 512], dt.float32, name=f"z{g}", tag=f"z{g}")
                for g in "gifo"
            ]

        def xp_matmuls(Z, s):
            # x@Wi enters PSUM via PE transpose of the gathered rows: these
            # matmuls need no h, so they overlap the previous step's tail.
            for t in range(NT):
                # start=True on each tile's first matmul clears that bank
                # region's has_written bits; later matmuls accumulate.
                nc.tensor.matmul(
                    out=Z[t // 4][:, 16 * (t % 4) : 16 * (t % 4) + 16],
                    lhsT=ST[s][:, 128 * t : 128 * t + 128],
                    rhs=id_sb[:],
                    start=(t % 4 == 0),
                    stop=False,
                )

        def step(iv, s, Z, Znext):
            hprev = H[(s - 1) % body]
            cprev = C[(s - 1) % body]
            # Gate order along z-features is (g, f, i, o), four tiles each.
            # Tile-outer / k-inner matmul order completes each gate's PSUM
            # slice early, so its sigmoid + DVE ops run while later gates'
            # matmuls still stream; only sigma_o || c-add -> tanh -> h remain
            # after the burst. The g block is pre-scaled x2 host-side so
            # tanh(g) = 2*sigmoid(2g) - 1 reuses the sigmoid table.
            S = sp.tile([128, 256], dt.float32, name="S", tag="S")
            TG = gp.tile([128, 64], dt.float16, name="TG", tag="TG")
            A = ap_.tile([128, 64], dt.float32, name="A", tag="A")
            B = bp.tile([128, 64], dt.float32, name="B", tag="B")
            T = tp.tile([128, 64], dt.float16, name="T", tag="T")
            So = s2p.tile([128, 64], dt.float16, name="So", tag="So")
            cs = C[s]
            for t in range(NT):
                for k in range(NK):
                    nc.tensor.matmul(
                        out=Z[t // 4][:, 16 * (t % 4) : 16 * (t % 4) + 16],
                        lhsT=wh_sb[:, (k * NT + t) * 128 : (k * NT + t) * 128 + 128],
                        rhs=hprev[:, 16 * k : 16 * k + 16],
                        start=False,
                        stop=(t % 4 == 3 and k == NK - 1),
                    )
                if t == 3:  # g tiles 0-3 complete
                    nc.scalar.activation(out=S[:, 0:64], in_=Z[0][:, 0:64], func=Act.Sigmoid)
                    nc.vector.tensor_scalar(out=TG[:], in0=S[:, 0:64], scalar1=2.0,
                                            scalar2=1.0, op0=Alu.mult, op1=Alu.subtract)
                elif t == 7:  # i tiles 4-7 complete
                    nc.scalar.activation(out=S[:, 64:128], in_=Z[1][:, 0:64], func=Act.Sigmoid)
                    nc.vector.tensor_tensor(out=A[:], in0=S[:, 64:128], in1=TG[:], op=Alu.mult)
                elif t == 11:  # f tiles 8-11 complete: c and tanh(c) still
                    # fit inside the o-gate matmuls. sigma_f stays fp32: its
                    # error compounds multiplicatively through c.
                    nc.scalar.activation(out=S[:, 128:192], in_=Z[2][:, 0:64], func=Act.Sigmoid)
                    nc.vector.tensor_tensor(out=B[:], in0=S[:, 128:192], in1=cprev[:], op=Alu.mult)
                    nc.vector.tensor_tensor(out=cs[:], in0=A[:], in1=B[:], op=Alu.add)
                    nc.scalar.activation(out=T[:], in_=cs[:], func=Act.Tanh)
            # Next step's x transposes go here in program order so the PE
            # runs them during this step's tail instead of at the head of the
            # next h-dependent burst.
            if Znext is not None:
                xp_matmuls(Znext, s + 1)
            # o tiles 12-15: only sigma_o -> h remains after the burst
            nc.scalar.activation(out=So[:], in_=Z[3][:, 0:64], func=Act.Sigmoid)
            hs = H[s]
            nc.vector.tensor_tensor(out=hs[:], in0=So[:], in1=T[:], op=Alu.mult)

            nc.sync.dma_start(out=c_traj[ds((iv + s) * 128, 128), :], in_=cs[:])
            nc.sync.dma_start(out=h_traj[ds((iv + s) * 128, 128), :], in_=hs[:])
            # Prefetch this slot's xp for the next block (the token table is
            # padded so the final block reads harmless extra rows).
            gather_xp(s)

        with tc.For_i(0, seq, body, hint_engines=(mybir.EngineType.PE,), staggered_reset=True) as iv:
            # Stage the NEXT block's token columns; in-loop gathers prefetch
            # for block i+1 while this block computes.
            nc.sync.dma_start(out=tok_cur[:], in_=tok[:, ds(iv + body, body)])
            Z = alloc_z()
            xp_matmuls(Z, 0)
            for s in range(body):
                Znext = alloc_z() if s < body - 1 else None
                step(iv, s, Z, Znext)
                Z = Znext

    nc.finalize()
    return nc


def _get_program():
    global _PROGRAM
    if _PROGRAM is None:
        _PROGRAM = _build_program()
    return _PROGRAM


def _prep_host(inputs, Wi, Wh, b):
    tokens = np.argmax(inputs, axis=-1).astype(np.int32)  # [B, T]
    eos = inputs[:, :, EOS_ID] > 0.5
    any_eos = eos.any(axis=1)
    t_star = np.where(any_eos, eos.argmax(axis=1), SEQ - 1).astype(np.int64)

    # Gate reorder (g, f, i, o): each gate's four z-feature tiles finish
    # early in the tile-outer matmul order, overlapping its tail ops with
    # the remaining matmuls. The g block is pre-scaled x2 so
    # tanh(g) = 2*sigmoid(2g) - 1 reuses the sigmoid table.
    perm = np.concatenate(
        [np.arange(1024, 1536), np.arange(0, 512), np.arange(512, 1024), np.arange(1536, 2048)]
    )
    gate_scale = np.ones((GATES,), np.float32)
    gate_scale[0:512] = 2.0  # g block after permutation
    Wi_re = (Wi.astype(np.float32) + b.astype(np.float32)[None, :])[:, perm] * gate_scale
    Wh_re = Wh.astype(np.float32)[:, perm] * gate_scale

    Wi_dev = np.ascontiguousarray(Wi_re).astype(np.float16)
    # Partition-major: wh[kr, (k*NT+t)*128 + p] = Wh_re[128k+kr, 128t+p]
    Wh_dev = np.ascontiguousarray(
        Wh_re.reshape(NK, 128, NT, 128).transpose(1, 0, 2, 3).reshape(128, NK * NT * 128)
    ).astype(np.float16)
    return tokens, t_star, Wi_dev, Wh_dev


def kernel(inputs, Wi, Wh, b):
    global LAST_RESULTS
    inputs = np.asarray(inputs)
    Wi = np.asarray(Wi)
    Wh = np.asarray(Wh)
    b = np.asarray(b)

    tokens, t_star, Wi_dev, Wh_dev = _prep_host(inputs, Wi, Wh, b)

    in_maps = []
    for n in range(NCORES):
        tokc = tokens[BLOC * n : BLOC * (n + 1)]
        tok_pad = np.concatenate([tokc, np.zeros((BLOC, BODY), np.int32)], axis=1)
        in_maps.append(
            {
                "wi": Wi_dev,
                "wh": Wh_dev,
                "tok": np.ascontiguousarray(tok_pad),
                "ident": np.eye(BLOC, dtype=np.float16),
            }
        )

    nc = _get_program()
    res = run_bass_kernel_spmd(nc, in_maps, list(range(NCORES)), trace=TRACE)
    LAST_RESULTS = res

    c_out = np.zeros((BATCH, HID), np.float32)
    h_out = np.zeros((BATCH, HID), np.float32)
    for n in range(NCORES):
        ct = res.results[n]["c_traj"].reshape(SEQ, 128, 64)
        ht = res.results[n]["h_traj"].reshape(SEQ, 128, 64).astype(np.float32)
        for bl in range(BLOC):
            g = BLOC * n + bl
            t = int(t_star[g])
            c_out[g] = ct[t][:, bl::BLOC].T.reshape(HID)
            h_out[g] = ht[t][:, bl::BLOC].T.reshape(HID)
    return (c_out, h_out)



# revision 30
# speedup vs baseline: 1.0103x; 1.0080x over previous
"""LSTM encoder with EOS-freeze for Trainium2, data-parallel over batch on 8 cores.

Strategy
--------
Inputs are one-hot, so x @ Wi is a row-gather of Wi done with indirect DMA on
device. The recurrent h @ Wh runs on the tensor engine with Wh as 64 fp16
[128,128] stationary tiles and h.T chunks as the [128,16] moving operand,
producing z transposed: per-gate PSUM banks [128 partitions = feature % 128,
16*tile + b].

The per-step critical path is (h-dependent matmul burst) -> (gate tail) -> h.
To hide the tail: gates are reordered (g, i, f, o) host-side and the matmuls
run tile-outer, so each gate's sigmoid + DVE ops overlap the later gates'
matmuls; each gate gets a full PSUM bank so those reads never serialize the
stream. The g block is pre-scaled by 2 so tanh(g) = 2*sigmoid(2g) - 1 stays
on the sigmoid table, and only sigma_o || (c = A+B) -> tanh(c) -> h remain
after the burst. Next-step x-transposes are emitted before this step's tail
so the PE runs them in the tail window.

The EOS freeze is handled without any per-step masking: sequences are
independent, so the kernel runs the unmasked recurrence and streams per-step
(c, h) snapshots to DRAM; the frozen value for sequence b is the snapshot at
its first-EOS step, selected during unshard.
"""

import numpy as np

try:
    import concourse  # noqa: F401
except ImportError:
    import sys

    sys.path.insert(0, "/opt/trn_rl_repo")

from contextlib import ExitStack

import concourse.bass as bass
import concourse.tile as tile
from concourse import bacc
from concourse import mybir
from concourse.bass import ds
from concourse.bass_utils import run_bass_kernel_spmd

dt = mybir.dt
Alu = mybir.AluOpType
Act = mybir.ActivationFunctionType

EOS_ID = 1
HID = 512
BATCH, SEQ, VOCAB = 128, 256, 1024
GATES = 4 * HID  # 2048
NCORES = 8
BLOC = BATCH // NCORES  # 16 sequences per core
NT = GATES // 128  # 16 feature tiles of z
NK = HID // 128  # 4 contraction chunks
BODY = 16  # steps per For_i iteration

# Collect profiling info when True (set by test.py; adds trace overhead).
TRACE = False
LAST_RESULTS = None  # BassKernelResults of the last run, for test.py

_PROGRAM = None


def _build_program(seq=SEQ, body=BODY):
    nc = bacc.Bacc("TRN2", debug=False, detect_race_conditions=False)

    wi = nc.declare_dram_parameter("wi", [VOCAB, GATES], dt.float16, isOutput=False)
    ident = nc.declare_dram_parameter("ident", [BLOC, BLOC], dt.float16, isOutput=False)
    wh = nc.declare_dram_parameter("wh", [128, NK * NT * 128], dt.float16, isOutput=False)
    tok = nc.declare_dram_parameter("tok", [BLOC, seq + body], dt.int32, isOutput=False)
    c_traj = nc.declare_dram_parameter("c_traj", [seq * 128, 64], dt.float32, isOutput=True)
    h_traj = nc.declare_dram_parameter("h_traj", [seq * 128, 64], dt.float16, isOutput=True)

    with tile.TileContext(nc) as tc, ExitStack() as ctx:
        pool = lambda name, bufs, **kw: ctx.enter_context(
            tc.tile_pool(name=name, bufs=bufs, **kw)
        )
        whp = pool("whp", 1)
        tokp = pool("tokp", 1)
        stp = pool("stp", 1)
        hp = pool("hp", 1)
        cp = pool("cp", 1)
        zp_pool = pool("zp", 2, space="PSUM")
        sp = pool("sp", 3)
        gp = pool("gp", 3)
        ap_ = pool("ap", 3)
        bp = pool("bp", 3)
        s2p = pool("s2p", 2)
        tp = pool("tp", 3)

        wh_sb = whp.tile([128, NK * NT * 128], dt.float16, name="wh_sb")
        nc.sync.dma_start(out=wh_sb[:], in_=wh[:, :])
        tok_cur = tokp.tile([BLOC, body], dt.int32, name="tok_cur")
        nc.sync.dma_start(out=tok_cur[:], in_=tok[:, 0:body])
        id_sb = tokp.tile([BLOC, BLOC], dt.float16, name="id_sb")
        nc.sync.dma_start(out=id_sb[:], in_=ident[:, :])

        ST = [stp.tile([BLOC, GATES], dt.float16, name=f"st{s}", tag=f"st{s}") for s in range(body)]
        H = [hp.tile([128, 64], dt.float16, name=f"h{s}", tag=f"h{s}") for s in range(body)]
        C = [cp.tile([128, 64], dt.float32, name=f"c{s}", tag=f"c{s}") for s in range(body)]

        nc.gpsimd.memset(H[body - 1][:], 0.0)
        nc.gpsimd.memset(C[body - 1][:], 0.0)
        for s in range(body):
            # init shadow coverage; real values come from the indirect gathers
            nc.gpsimd.memset(ST[s][:], 0.0)

        def gather_xp(s):
            # Gather BLOC wi rows (one per sequence) for one timestep into
            # ST[s][b, :] — row-per-partition, the DGE-supported shape.
            # tok_cur always holds the token column for the block being
            # prefetched, so the offset AP stays static.
            nc.gpsimd.indirect_dma_start(
                out=ST[s][:],
                out_offset=None,
                in_=wi[:, :],
                in_offset=bass.IndirectOffsetOnAxis(ap=tok_cur[:, s : s + 1], axis=0),
            )

        for s in range(body):
            gather_xp(s)

        # Touch the sigmoid table before the loop so the act-table placement
        # sees it loaded on every path into the loop body (otherwise each
        # iteration pays a ~1.3us ACT_TABLE_LOAD at entry).
        warm = tokp.tile([128, 16], dt.float32, name="warm")
        nc.gpsimd.memset(warm[:], 0.0)
        nc.scalar.activation(out=warm[:], in_=warm[:], func=Act.Sigmoid)
        nc.scalar.activation(out=warm[:], in_=warm[:], func=Act.Tanh)

        def alloc_z():
            # One full PSUM bank per gate (a tile smaller than a bank shares
            # it, and start=True clears whole banks, which would serialize
            # steps): a finished gate's sigmoid read never blocks the
            # still-streaming matmuls of later gates.
            return [
                zp_pool.tile([128, 512], dt.float32, name=f"z{g}", tag=f"z{g}")
                for g in "gifo"
            ]

        def xp_matmuls(Z, s):
            # x@Wi enters PSUM via PE transpose of the gathered rows: these
            # matmuls need no h, so they overlap the previous step's tail.
            for t in range(NT):
                # start=True on each tile's first matmul clears that bank
                # region's has_written bits; later matmuls accumulate.
                nc.tensor.matmul(
                    out=Z[t // 4][:, 16 * (t % 4) : 16 * (t % 4) + 16],
                    lhsT=ST[s][:, 128 * t : 128 * t + 128],
                    rhs=id_sb[:],
                    start=(t % 4 == 0),
                    stop=False,
                )

        def step(iv, s, Z, Znext):
            hprev = H[(s - 1) % body]
            cprev = C[(s - 1) % body]
            # Gate order along z-features is (g, f, i, o), four tiles each.
            # Tile-outer / k-inner matmul order completes each gate's PSUM
            # slice early, so its sigmoid + DVE ops run while later gates'
            # matmuls still stream; only sigma_o || c-add -> tanh -> h remain
            # after the burst. The g block is pre-scaled x2 host-side so
            # tanh(g) = 2*sigmoid(2g) - 1 reuses the sigmoid table.
            S = sp.tile([128, 256], dt.float32, name="S", tag="S")
            TG = gp.tile([128, 64], dt.float16, name="TG", tag="TG")
            A = ap_.tile([128, 64], dt.float32, name="A", tag="A")
            B = bp.tile([128, 64], dt.float32, name="B", tag="B")
            T = tp.tile([128, 64], dt.float16, name="T", tag="T")
            So = s2p.tile([128, 64], dt.float16, name="So", tag="So")
            cs = C[s]
            for t in range(NT):
                for k in range(NK):
                    nc.tensor.matmul(
                        out=Z[t // 4][:, 16 * (t % 4) : 16 * (t % 4) + 16],
                        lhsT=wh_sb[:, (k * NT + t) * 128 : (k * NT + t) * 128 + 128],
                        rhs=hprev[:, 16 * k : 16 * k + 16],
                        start=False,
                        stop=(t % 4 == 3 and k == NK - 1),
                    )
                if t == 3:  # g tiles 0-3 complete
                    nc.scalar.activation(out=S[:, 0:64], in_=Z[0][:, 0:64], func=Act.Sigmoid)
                    nc.vector.tensor_scalar(out=TG[:], in0=S[:, 0:64], scalar1=2.0,
                                            scalar2=1.0, op0=Alu.mult, op1=Alu.subtract)
                elif t == 7:  # i tiles 4-7 complete
                    nc.scalar.activation(out=S[:, 64:128], in_=Z[1][:, 0:64], func=Act.Sigmoid)
                    nc.vector.tensor_tensor(out=A[:], in0=S[:, 64:128], in1=TG[:], op=Alu.mult)
                elif t == 11:  # f tiles 8-11 complete: c and tanh(c) still
                    # fit inside the o-gate matmuls. sigma_f stays fp32: its
                    # error compounds multiplicatively through c.
                    nc.scalar.activation(out=S[:, 128:192], in_=Z[2][:, 0:64], func=Act.Sigmoid)
                    nc.vector.tensor_tensor(out=B[:], in0=S[:, 128:192], in1=cprev[:], op=Alu.mult)
                    nc.vector.tensor_tensor(out=cs[:], in0=A[:], in1=B[:], op=Alu.add)
                    nc.scalar.activation(out=T[:], in_=cs[:], func=Act.Tanh)
            # Next step's x transposes go here in program order so the PE
            # runs them during this step's tail instead of at the head of the
            # next h-dependent burst.
            if Znext is not None:
                xp_matmuls(Znext, s + 1)
            # o tiles 12-15: only sigma_o -> h remains after the burst
            nc.scalar.activation(out=So[:], in_=Z[3][:, 0:64], func=Act.Sigmoid)
            hs = H[s]
            nc.vector.tensor_tensor(out=hs[:], in0=So[:], in1=T[:], op=Alu.mult)

            nc.sync.dma_start(out=c_traj[ds((iv + s) * 128, 128), :], in_=cs[:])
            nc.sync.dma_start(out=h_traj[ds((iv + s) * 128, 128), :], in_=hs[:])
            # Prefetch this slot's xp for the next block (the token table is
            # padded so the final block reads harmless extra rows).
            gather_xp(s)

        with tc.For_i(0, seq, body, hint_engines=(mybir.EngineType.PE,), staggered_reset=True) as iv:
            # Stage the NEXT block's token columns; in-loop gathers prefetch
            # for block i+1 while this block computes.
            nc.sync.dma_start(out=tok_cur[:], in_=tok[:, ds(iv + body, body)])
            Z = alloc_z()
            xp_matmuls(Z, 0)
            for s in range(body):
                Znext = alloc_z() if s < body - 1 else None
                step(iv, s, Z, Znext)
                Z = Znext

    nc.finalize()
    return nc


def _get_program():
    global _PROGRAM
    if _PROGRAM is None:
        _PROGRAM = _build_program()
    return _PROGRAM


def _prep_host(inputs, Wi, Wh, b):
    tokens = np.argmax(inputs, axis=-1).astype(np.int32)  # [B, T]
    eos = inputs[:, :, EOS_ID] > 0.5
    any_eos = eos.any(axis=1)
    t_star = np.where(any_eos, eos.argmax(axis=1), SEQ - 1).astype(np.int64)

    # Gate reorder (g, f, i, o): each gate's four z-feature tiles finish
    # early in the tile-outer matmul order, overlapping its tail ops with
    # the remaining matmuls. The g block is pre-scaled x2 so
    # tanh(g) = 2*sigmoid(2g) - 1 reuses the sigmoid table.
    perm = np.concatenate(
        [np.arange(1024, 1536), np.arange(0, 512), np.arange(512, 1024), np.arange(1536, 2048)]
    )
    gate_scale = np.ones((GATES,), np.float32)
    gate_scale[0:512] = 2.0  # g block after permutation
    Wi_re = (Wi.astype(np.float32) + b.astype(np.float32)[None, :])[:, perm] * gate_scale
    Wh_re = Wh.astype(np.float32)[:, perm] * gate_scale

    Wi_dev = np.ascontiguousarray(Wi_re).astype(np.float16)
    # Partition-major: wh[kr, (k*NT+t)*128 + p] = Wh_re[128k+kr, 128t+p]
    Wh_dev = np.ascontiguousarray(
        Wh_re.reshape(NK, 128, NT, 128).transpose(1, 0, 2, 3).reshape(128, NK * NT * 128)
    ).astype(np.float16)
    return tokens, t_star, Wi_dev, Wh_dev


def kernel(inputs, Wi, Wh, b):
    global LAST_RESULTS
    inputs = np.asarray(inputs)
    Wi = np.asarray(Wi)
    Wh = np.asarray(Wh)
    b = np.asarray(b)

    tokens, t_star, Wi_dev, Wh_dev = _prep_host(inputs, Wi, Wh, b)

    in_maps = []
    for n in range(NCORES):
        tokc = tokens[BLOC * n : BLOC * (n + 1)]
        tok_pad = np.concatenate([tokc, np.zeros((BLOC, BODY), np.int32)], axis=1)
        in_maps.append(
            {
                "wi": Wi_dev,
                "wh": Wh_dev,
                "tok": np.ascontiguousarray(tok_pad),
                "ident": np.eye(BLOC, dtype=np.float16),
            }
        )

    nc = _get_program()
    res = run_bass_kernel_spmd(nc, in_maps, list(range(NCORES)), trace=TRACE)
    LAST_RESULTS = res

    c_out = np.zeros((BATCH, HID), np.float32)
    h_out = np.zeros((BATCH, HID), np.float32)
    for n in range(NCORES):
        ct = res.results[n]["c_traj"].reshape(SEQ, 128, 64)
        ht = res.results[n]["h_traj"].reshape(SEQ, 128, 64).astype(np.float32)
        for bl in range(BLOC):
            g = BLOC * n + bl
            t = int(t_star[g])
            c_out[g] = ct[t][:, bl::BLOC].T.reshape(HID)
            h_out[g] = ht[t][:, bl::BLOC].T.reshape(HID)
    return (c_out, h_out)

